# revision 3
# baseline (speedup 1.0000x reference)
"""CIF (continuous integrate-and-fire) kernel for Trainium2, 8 NeuronCores.

Strategy
--------
The CIF scan over time only has a *scalar* recurrence: the integrate/fire
decisions and the per-step blend weights depend solely on ``alphas`` [B, T]
(256 KB).  All the heavy work involving ``hidden`` [B, T, H] (131 MB) is,
for fixed fire decisions, a linear map: every output frame j is a weighted
sum of consecutive hidden rows,

    out[b, j, :] = sum_t WT[b, t, j] * hidden[b, t, :]

where WT[b] is a [T, 64] sparse-banded weight matrix (each time step
contributes to at most two adjacent frames; weights are the reference's
``cur``/``remainds`` values).

So: replicate the reference's fp32 scalar scan on the host (exact same op
order -> bit-identical fire decisions), build WT, then run the batched
[64, T] @ [T, H] matmul on the 8 NeuronCores — pure data parallel over the
batch dim, 4 rows per core, K-tiled over T with PSUM accumulation.
"""

import os
import sys

import numpy as np

# --- problem constants (hardcoded per spec: nn_CIF_Model_5970004541927) ---
B, T, H = 32, 2000, 512
NCORES = 8
R = B // NCORES          # batch rows per core = 4
ML = 64                  # MAX_LABELS
THRESH = np.float32(0.95)
P = 128                  # SBUF partitions
NFULL = T // P           # 15 full K-chunks
TAIL = T - NFULL * P     # 80 leftover time steps
NCHUNK = NFULL + 1       # 16
TP = NCHUNK * P          # 2048 (weights padded so chunks divide evenly)

# matmul input dtype on the PE: "fp32" (exact, 4 cycles/row) or
# "fp32r" (TF32-like fast path, 1 cycle/row at N>=256)
MM_MODE = os.environ.get("CIF_MM_MODE", "fp32r")

_COMPILED = {}


def _build_weights(alphas: np.ndarray) -> np.ndarray:
    """Replicate the reference fp32 scan on alphas only; return WT [B, TP, ML].

    Per time step t (exactly the reference ops, vectorized over the batch):
        dist_completion = 1 - integrate
        integrate += a_t ; fire = integrate > 0.95
        integrate -= fire
        cur = fire ? dist_completion : a_t          -> weight into frame n (n = fires so far)
        remainds = a_t - cur                        -> at a fire, leftover into frame n+1
    Frame j's recorded value is frame_out at the j-th fire, i.e. the sum of
    these contributions, so WT columns hold cur/remainds at the right rows.
    """
    Bv, Tv = alphas.shape
    a = np.ascontiguousarray(alphas, dtype=np.float32)
    integrate = np.zeros(Bv, np.float32)
    nfires = np.zeros(Bv, np.int64)
    # two dump columns absorb contributions past frame ML-1
    WT = np.zeros((Bv, TP, ML + 2), np.float32)
    rows = np.arange(Bv)
    one = np.float32(1.0)
    for t in range(Tv):
        a_t = a[:, t]
        dist_completion = one - integrate
        integrate = integrate + a_t
        fire = integrate > THRESH
        integrate = np.where(fire, integrate - one, integrate)
        cur = np.where(fire, dist_completion, a_t)
        remainds = a_t - cur
        j = np.minimum(nfires, ML)
        WT[rows, t, j] = cur
        if fire.any():
            fr = rows[fire]
            j2 = np.minimum(nfires[fire] + 1, ML + 1)
            WT[fr, t, j2] = remainds[fire]
        nfires = nfires + fire
    return np.ascontiguousarray(WT[:, :, :ML])


def _build_nc(mm_mode: str):
    """Emit the Bass/Tile program (identical on all 8 cores; SPMD over batch)."""
    import concourse.bacc as bacc
    import concourse.mybir as mybir
    import concourse.tile as tile

    f32 = mybir.dt.float32
    mm_dt = {"fp32": f32, "fp32r": mybir.dt.float32r, "bf16": mybir.dt.bfloat16}[
        mm_mode
    ]
    in_dt = mybir.dt.bfloat16 if mm_mode == "bf16" else f32

    nc = bacc.Bacc("TRN2", target_bir_lowering=False, debug=False)
    hid = nc.dram_tensor("hidden", [R, T, H], in_dt, kind="ExternalInput")
    wt = nc.dram_tensor("wt", [R, TP, ML], in_dt, kind="ExternalInput")
    out = nc.dram_tensor("out", [R, ML, H], f32, kind="ExternalOutput")

    with tile.TileContext(nc) as tc:
        with (
            tc.tile_pool(name="hpool", bufs=2) as hpool,
            tc.tile_pool(name="wpool", bufs=2) as wpool,
            tc.tile_pool(name="opool", bufs=2) as opool,
            tc.tile_pool(name="psum", bufs=2, space="PSUM") as psum_pool,
        ):
            for r in range(R):
                # weights for this row: [TP, ML] -> [P, NCHUNK, ML]
                w_tile = wpool.tile([P, NCHUNK * ML], in_dt, tag="w")
                nc.sync.dma_start(
                    w_tile[:].rearrange("p (c m) -> p c m", c=NCHUNK),
                    wt[r].rearrange("(c p) m -> p c m", p=P),
                )
                # hidden rows 0..1919: [1920, H] -> [P, NFULL, H]
                h_main = hpool.tile([P, NFULL * H], in_dt, tag="h")
                nc.sync.dma_start(
                    h_main[:].rearrange("p (c h) -> p c h", c=NFULL),
                    hid[r, 0 : NFULL * P].rearrange("(c p) h -> p c h", p=P),
                )
                # hidden tail rows 1920..1999 -> [TAIL, H]
                h_tail = hpool.tile([P, H], in_dt, tag="ht")
                nc.sync.dma_start(h_tail[0:TAIL, :], hid[r, NFULL * P : T])

                ps = psum_pool.tile([ML, H], f32)
                for c in range(NFULL):
                    nc.tensor.matmul(
                        ps[:],
                        w_tile[:, c * ML : (c + 1) * ML].bitcast(mm_dt),
                        h_main[:, c * H : (c + 1) * H].bitcast(mm_dt),
                        start=(c == 0),
                        stop=False,
                    )
                nc.tensor.matmul(
                    ps[:],
                    w_tile[0:TAIL, NFULL * ML : NCHUNK * ML].bitcast(mm_dt),
                    h_tail[0:TAIL, :].bitcast(mm_dt),
                    start=False,
                    stop=True,
                )
                o_tile = opool.tile([ML, H], f32, tag="o")
                nc.vector.tensor_copy(o_tile[:], ps[:])
                nc.sync.dma_start(out[r], o_tile[:])
    nc.compile()
    return nc


def _get_nc(mm_mode: str):
    if mm_mode not in _COMPILED:
        _COMPILED[mm_mode] = _build_nc(mm_mode)
    return _COMPILED[mm_mode]


def kernel(hidden: np.ndarray, alphas: np.ndarray, _trace: bool = False):
    from concourse.bass_utils import run_bass_kernel_spmd

    hidden = np.ascontiguousarray(np.asarray(hidden, dtype=np.float32))
    alphas = np.asarray(alphas, dtype=np.float32)
    assert hidden.shape == (B, T, H) and alphas.shape == (B, T)

    WT = _build_weights(alphas)  # [B, TP, ML] fp32

    if MM_MODE == "bf16":
        import ml_dtypes

        hidden_dev = hidden.astype(ml_dtypes.bfloat16)
        wt_dev = WT.astype(ml_dtypes.bfloat16)
    else:
        hidden_dev = hidden
        wt_dev = WT

    nc = _get_nc(MM_MODE)
    in_maps = [
        {
            "hidden": hidden_dev[c * R : (c + 1) * R],
            "wt": wt_dev[c * R : (c + 1) * R],
        }
        for c in range(NCORES)
    ]
    res = run_bass_kernel_spmd(nc, in_maps, list(range(NCORES)), trace=_trace)
    out = np.concatenate([res.results[c]["out"] for c in range(NCORES)], axis=0)
    out = np.ascontiguousarray(out.astype(np.float32))
    if _trace:
        return out, res
    return out


# revision 4
# speedup vs baseline: 1.1144x; 1.1144x over previous
"""CIF (continuous integrate-and-fire) kernel for Trainium2, 8 NeuronCores.

Strategy
--------
The CIF scan over time only has a *scalar* recurrence: the integrate/fire
decisions and the per-step blend weights depend solely on ``alphas`` [B, T]
(256 KB).  All the heavy work involving ``hidden`` [B, T, H] (131 MB) is,
for fixed fire decisions, a linear map: every output frame j is a weighted
sum of consecutive hidden rows,

    out[b, j, :] = sum_t W[b, t, j] * hidden[b, t, :]

where W[b] is a [T, 64] sparse-banded weight matrix (each time step
contributes to at most two adjacent frames; weights are the reference's
``cur``/``remainds`` values).

So: replicate the reference's fp32 scalar scan on the host (exact same op
order -> bit-identical fire decisions), build W, then run the batched
[64, T] @ [T, H] matmul on the 8 NeuronCores — pure data parallel over the
batch dim, 4 rows per core, K-tiled over T with PSUM accumulation.
DMAs are issued in ~1 MB chunk-groups so the PE starts ~4 us in and the
transfer stream stays ahead of the matmuls.
"""

import os

import numpy as np

# --- problem constants (hardcoded per spec: nn_CIF_Model_5970004541927) ---
B, T, H = 32, 2000, 512
NCORES = 8
R = B // NCORES          # batch rows per core = 4
ML = 64                  # MAX_LABELS
THRESH = np.float32(0.95)
P = 128                  # SBUF partitions
NFULL = T // P           # 15 full K-chunks
TAIL = T - NFULL * P     # 80 leftover time steps
NCHUNK = NFULL + 1       # 16
TP = NCHUNK * P          # 2048 (weights padded so chunks divide evenly)
GRP = 4                  # K-chunks per hidden DMA (~1 MB fp32)

# matmul input dtype on the PE: "fp32" (exact, 4 cycles/row),
# "fp32r" (TF32-like fast path, 1 cycle/row at N>=256), or "bf16"
MM_MODE = os.environ.get("CIF_MM_MODE", "fp32")

_COMPILED = {}


def _build_weights(alphas: np.ndarray) -> np.ndarray:
    """Replicate the reference fp32 scan on alphas only.

    Returns WF [B, P, NCHUNK, ML] float32 — the lhsT tiles laid out so the
    device DMA reads one contiguous 4 KB run per partition:
    WF[b, p, c, m] = weight of hidden step t = c*P + p into output frame m.

    Per time step t (exactly the reference ops, vectorized over the batch):
        dist_completion = 1 - integrate
        integrate += a_t ; fire = integrate > 0.95
        integrate -= fire
        cur = fire ? dist_completion : a_t   -> frame n   (n = fires so far)
        remainds = a_t - cur                 -> frame n+1  (only at a fire)
    """
    Bv, Tv = alphas.shape
    a = np.ascontiguousarray(alphas, dtype=np.float32)
    integrate = np.zeros(Bv, np.float32)
    nfires = np.zeros(Bv, np.int64)
    # two dump columns absorb contributions past frame ML-1
    WT = np.zeros((Bv, TP, ML + 2), np.float32)
    rows = np.arange(Bv)
    one = np.float32(1.0)
    for t in range(Tv):
        a_t = a[:, t]
        dist_completion = one - integrate
        integrate = integrate + a_t
        fire = integrate > THRESH
        integrate = np.where(fire, integrate - one, integrate)
        cur = np.where(fire, dist_completion, a_t)
        remainds = a_t - cur
        j = np.minimum(nfires, ML)
        WT[rows, t, j] = cur
        if fire.any():
            fr = rows[fire]
            j2 = np.minimum(nfires[fire] + 1, ML + 1)
            WT[fr, t, j2] = remainds[fire]
        nfires = nfires + fire
    WT = WT[:, :, :ML]                                  # [B, TP, ML]
    WF = WT.reshape(Bv, NCHUNK, P, ML).transpose(0, 2, 1, 3)  # [B, P, NCHUNK, ML]
    return np.ascontiguousarray(WF)


def _build_nc(mm_mode: str):
    """Emit the Bass/Tile program (identical on all 8 cores; SPMD over batch)."""
    import concourse.bacc as bacc
    import concourse.mybir as mybir
    import concourse.tile as tile

    f32 = mybir.dt.float32
    mm_dt = {"fp32": f32, "fp32r": mybir.dt.float32r, "bf16": mybir.dt.bfloat16}[
        mm_mode
    ]
    in_dt = mybir.dt.bfloat16 if mm_mode == "bf16" else f32

    nc = bacc.Bacc("TRN2", target_bir_lowering=False, debug=False)
    hid = nc.dram_tensor("hidden", [R, T, H], in_dt, kind="ExternalInput")
    wt = nc.dram_tensor("wt", [R, P, NCHUNK * ML], in_dt, kind="ExternalInput")
    out = nc.dram_tensor("out", [R, ML, H], f32, kind="ExternalOutput")

    # chunk-groups per row: GRP full chunks per DMA, tail chunk separate
    groups = [
        list(range(g, min(g + GRP, NFULL))) for g in range(0, NFULL, GRP)
    ]  # [[0..3],[4..7],[8..11],[12..14]]

    with tile.TileContext(nc) as tc:
        with (
            tc.tile_pool(name="hpool", bufs=8) as hpool,
            tc.tile_pool(name="wpool", bufs=2) as wpool,
            tc.tile_pool(name="opool", bufs=2) as opool,
            tc.tile_pool(name="psum", bufs=2, space="PSUM") as psum_pool,
        ):
            # alternate HWDGE queues for parallel descriptor generation
            dma_engines = [nc.sync, nc.scalar]

            for r in range(R):
                di = 0
                # weights for this row: one contiguous 4 KB run per partition
                w_tile = wpool.tile([P, NCHUNK * ML], in_dt, tag="w")
                dma_engines[di % 2].dma_start(w_tile[:], wt[r])
                di += 1

                h_tiles = []
                for gi, g in enumerate(groups):
                    n = len(g)
                    ht = hpool.tile([P, GRP * H], in_dt, tag="h")
                    dma_engines[di % 2].dma_start(
                        ht[:].rearrange("p (c h) -> p c h", c=GRP)[:, :n],
                        hid[r, g[0] * P : (g[-1] + 1) * P].rearrange(
                            "(c p) h -> p c h", p=P
                        ),
                    )
                    di += 1
                    h_tiles.append(ht)
                h_tail = hpool.tile([P, H], in_dt, tag="ht")
                dma_engines[di % 2].dma_start(h_tail[0:TAIL, :], hid[r, NFULL * P : T])

                ps = psum_pool.tile([ML, H], f32)
                for gi, g in enumerate(groups):
                    ht = h_tiles[gi]
                    for ci, c in enumerate(g):
                        nc.tensor.matmul(
                            ps[:],
                            w_tile[:, c * ML : (c + 1) * ML].bitcast(mm_dt),
                            ht[:, ci * H : (ci + 1) * H].bitcast(mm_dt),
                            start=(c == 0),
                            stop=False,
                        )
                nc.tensor.matmul(
                    ps[:],
                    w_tile[0:TAIL, NFULL * ML : NCHUNK * ML].bitcast(mm_dt),
                    h_tail[0:TAIL, :].bitcast(mm_dt),
                    start=False,
                    stop=True,
                )
                o_tile = opool.tile([ML, H], f32, tag="o")
                nc.vector.tensor_copy(o_tile[:], ps[:])
                nc.sync.dma_start(out[r], o_tile[:])
    nc.compile()
    return nc


def _get_nc(mm_mode: str):
    if mm_mode not in _COMPILED:
        _COMPILED[mm_mode] = _build_nc(mm_mode)
    return _COMPILED[mm_mode]


def kernel(hidden: np.ndarray, alphas: np.ndarray, _trace: bool = False):
    from concourse.bass_utils import run_bass_kernel_spmd

    hidden = np.ascontiguousarray(np.asarray(hidden, dtype=np.float32))
    alphas = np.asarray(alphas, dtype=np.float32)
    assert hidden.shape == (B, T, H) and alphas.shape == (B, T)

    WF = _build_weights(alphas)  # [B, P, NCHUNK, ML] fp32

    if MM_MODE == "bf16":
        import ml_dtypes

        hidden_dev = hidden.astype(ml_dtypes.bfloat16)
        wt_dev = WF.astype(ml_dtypes.bfloat16)
    else:
        hidden_dev = hidden
        wt_dev = WF
    wt_dev = wt_dev.reshape(B, P, NCHUNK * ML)

    nc = _get_nc(MM_MODE)
    in_maps = [
        {
            "hidden": hidden_dev[c * R : (c + 1) * R],
            "wt": wt_dev[c * R : (c + 1) * R],
        }
        for c in range(NCORES)
    ]
    res = run_bass_kernel_spmd(nc, in_maps, list(range(NCORES)), trace=_trace)
    out = np.concatenate([res.results[c]["out"] for c in range(NCORES)], axis=0)
    out = np.ascontiguousarray(out.astype(np.float32))
    if _trace:
        return out, res
    return out


# revision 6
# speedup vs baseline: 1.3696x; 1.2289x over previous
"""CIF (continuous integrate-and-fire) kernel for Trainium2, 8 NeuronCores.

Strategy
--------
The CIF scan over time only has a *scalar* recurrence: the integrate/fire
decisions and the per-step blend weights depend solely on ``alphas`` [B, T]
(256 KB).  All the heavy work involving ``hidden`` [B, T, H] (131 MB) is,
for fixed fire decisions, a linear map: every output frame j is a weighted
sum of consecutive hidden rows,

    out[b, j, :] = sum_t W[b, t, j] * hidden[b, t, :]

where W[b] is a [T, 64] sparse-banded weight matrix (each time step
contributes to at most two adjacent frames; weights are the reference's
``cur``/``remainds`` values).

So: replicate the reference's fp32 scalar scan on the host (exact same op
order -> bit-identical fire decisions), build W, then run the batched
[64, T] @ [T, H] matmul on the 8 NeuronCores — pure data parallel over the
batch dim, 4 rows per core, K-tiled over T with PSUM accumulation.
DMAs are issued in ~1 MB chunk-groups so the PE starts ~4 us in and the
transfer stream stays ahead of the matmuls.
"""

import os

import numpy as np

# --- problem constants (hardcoded per spec: nn_CIF_Model_5970004541927) ---
B, T, H = 32, 2000, 512
NCORES = 8
R = B // NCORES          # batch rows per core = 4
ML = 64                  # MAX_LABELS
THRESH = np.float32(0.95)
P = 128                  # SBUF partitions
NFULL = T // P           # 15 full K-chunks
TAIL = T - NFULL * P     # 80 leftover time steps
NCHUNK = NFULL + 1       # 16
TP = NCHUNK * P          # 2048 (weights padded so chunks divide evenly)
GRP = 4                  # K-chunks per hidden DMA (~1 MB fp32)

# matmul input dtype on the PE: "fp32" (exact, 4 cycles/row),
# "fp32r" (TF32-like fast path, 1 cycle/row at N>=256), or "bf16"
MM_MODE = os.environ.get("CIF_MM_MODE", "fp32")

_COMPILED = {}


def _build_weights(alphas: np.ndarray) -> np.ndarray:
    """Replicate the reference fp32 scan on alphas only.

    Returns WF [B, P, NCHUNK, ML] float32 — the lhsT tiles laid out so the
    device DMA reads one contiguous 4 KB run per partition:
    WF[b, p, c, m] = weight of hidden step t = c*P + p into output frame m.

    Per time step t (exactly the reference ops, vectorized over the batch):
        dist_completion = 1 - integrate
        integrate += a_t ; fire = integrate > 0.95
        integrate -= fire
        cur = fire ? dist_completion : a_t   -> frame n   (n = fires so far)
        remainds = a_t - cur                 -> frame n+1  (only at a fire)
    """
    Bv, Tv = alphas.shape
    a = np.ascontiguousarray(alphas, dtype=np.float32)
    integrate = np.zeros(Bv, np.float32)
    nfires = np.zeros(Bv, np.int64)
    # two dump columns absorb contributions past frame ML-1
    WT = np.zeros((Bv, TP, ML + 2), np.float32)
    rows = np.arange(Bv)
    one = np.float32(1.0)
    for t in range(Tv):
        a_t = a[:, t]
        dist_completion = one - integrate
        integrate = integrate + a_t
        fire = integrate > THRESH
        integrate = np.where(fire, integrate - one, integrate)
        cur = np.where(fire, dist_completion, a_t)
        remainds = a_t - cur
        j = np.minimum(nfires, ML)
        WT[rows, t, j] = cur
        if fire.any():
            fr = rows[fire]
            j2 = np.minimum(nfires[fire] + 1, ML + 1)
            WT[fr, t, j2] = remainds[fire]
        nfires = nfires + fire
    WT = WT[:, :, :ML]                                  # [B, TP, ML]
    WF = WT.reshape(Bv, NCHUNK, P, ML).transpose(0, 2, 1, 3)  # [B, P, NCHUNK, ML]
    return np.ascontiguousarray(WF)


def _build_nc(mm_mode: str):
    """Emit the Bass/Tile program (identical on all 8 cores; SPMD over batch)."""
    import concourse.bacc as bacc
    import concourse.mybir as mybir
    import concourse.tile as tile

    f32 = mybir.dt.float32
    mm_dt = {"fp32": f32, "fp32r": mybir.dt.float32r, "bf16": mybir.dt.bfloat16}[
        mm_mode
    ]
    # fp32r: walrus requires matmul operands to be *produced* as float32r,
    # so declare the DRAM tensors and SBUF tiles as float32r throughout.
    in_dt = mm_dt

    nc = bacc.Bacc("TRN2", target_bir_lowering=False, debug=False)
    hid = nc.dram_tensor("hidden", [R, T, H], in_dt, kind="ExternalInput")
    wt = nc.dram_tensor("wt", [R, P, NCHUNK * ML], in_dt, kind="ExternalInput")
    out = nc.dram_tensor("out", [R, ML, H], f32, kind="ExternalOutput")

    # chunk-groups per row: GRP full chunks per DMA, tail chunk separate
    groups = [
        list(range(g, min(g + GRP, NFULL))) for g in range(0, NFULL, GRP)
    ]  # [[0..3],[4..7],[8..11],[12..14]]

    with tile.TileContext(nc) as tc:
        with (
            tc.tile_pool(name="hpool", bufs=8) as hpool,
            tc.tile_pool(name="wpool", bufs=2) as wpool,
            tc.tile_pool(name="opool", bufs=2) as opool,
            tc.tile_pool(name="psum", bufs=2, space="PSUM") as psum_pool,
        ):
            # alternate HWDGE queues for parallel descriptor generation
            dma_engines = [nc.sync, nc.scalar]

            for r in range(R):
                di = 0
                # weights for this row: one contiguous 4 KB run per partition
                w_tile = wpool.tile([P, NCHUNK * ML], in_dt, tag="w")
                dma_engines[di % 2].dma_start(w_tile[:], wt[r])
                di += 1

                h_tiles = []
                for gi, g in enumerate(groups):
                    n = len(g)
                    ht = hpool.tile([P, GRP * H], in_dt, tag="h")
                    dma_engines[di % 2].dma_start(
                        ht[:].rearrange("p (c h) -> p c h", c=GRP)[:, :n],
                        hid[r, g[0] * P : (g[-1] + 1) * P].rearrange(
                            "(c p) h -> p c h", p=P
                        ),
                    )
                    di += 1
                    h_tiles.append(ht)
                h_tail = hpool.tile([P, H], in_dt, tag="ht")
                dma_engines[di % 2].dma_start(h_tail[0:TAIL, :], hid[r, NFULL * P : T])

                ps = psum_pool.tile([ML, H], f32)
                for gi, g in enumerate(groups):
                    ht = h_tiles[gi]
                    for ci, c in enumerate(g):
                        nc.tensor.matmul(
                            ps[:],
                            w_tile[:, c * ML : (c + 1) * ML],
                            ht[:, ci * H : (ci + 1) * H],
                            start=(c == 0),
                            stop=False,
                        )
                nc.tensor.matmul(
                    ps[:],
                    w_tile[0:TAIL, NFULL * ML : NCHUNK * ML],
                    h_tail[0:TAIL, :],
                    start=False,
                    stop=True,
                )
                o_tile = opool.tile([ML, H], f32, tag="o")
                nc.vector.tensor_copy(o_tile[:], ps[:])
                nc.sync.dma_start(out[r], o_tile[:])
    nc.compile()
    return nc


def _get_nc(mm_mode: str):
    if mm_mode not in _COMPILED:
        _COMPILED[mm_mode] = _build_nc(mm_mode)
    return _COMPILED[mm_mode]


def kernel(hidden: np.ndarray, alphas: np.ndarray, _trace: bool = False):
    from concourse.bass_utils import run_bass_kernel_spmd

    hidden = np.ascontiguousarray(np.asarray(hidden, dtype=np.float32))
    alphas = np.asarray(alphas, dtype=np.float32)
    assert hidden.shape == (B, T, H) and alphas.shape == (B, T)

    WF = _build_weights(alphas)  # [B, P, NCHUNK, ML] fp32

    if MM_MODE == "bf16":
        import ml_dtypes

        hidden_dev = hidden.astype(ml_dtypes.bfloat16)
        wt_dev = WF.astype(ml_dtypes.bfloat16)
    else:
        hidden_dev = hidden
        wt_dev = WF
    wt_dev = wt_dev.reshape(B, P, NCHUNK * ML)

    nc = _get_nc(MM_MODE)
    in_maps = [
        {
            "hidden": hidden_dev[c * R : (c + 1) * R],
            "wt": wt_dev[c * R : (c + 1) * R],
        }
        for c in range(NCORES)
    ]
    res = run_bass_kernel_spmd(nc, in_maps, list(range(NCORES)), trace=_trace)
    out = np.concatenate([res.results[c]["out"] for c in range(NCORES)], axis=0)
    out = np.ascontiguousarray(out.astype(np.float32))
    if _trace:
        return out, res
    return out


# revision 7
# speedup vs baseline: 1.4807x; 1.0811x over previous
"""CIF (continuous integrate-and-fire) kernel for Trainium2, 8 NeuronCores.

Strategy
--------
The CIF scan over time only has a *scalar* recurrence: the integrate/fire
decisions and the per-step blend weights depend solely on ``alphas`` [B, T]
(256 KB).  All the heavy work involving ``hidden`` [B, T, H] (131 MB) is,
for fixed fire decisions, a linear map: every output frame j is a weighted
sum of consecutive hidden rows,

    out[b, j, :] = sum_t W[b, t, j] * hidden[b, t, :]

where W[b] is a [T, 64] sparse-banded weight matrix (each time step
contributes to at most two adjacent frames; weights are the reference's
``cur``/``remainds`` values).

So: replicate the reference's fp32 scalar scan on the host (exact same op
order -> bit-identical fire decisions), build W, then run the batched
[64, T] @ [T, H] matmul on the 8 NeuronCores — pure data parallel over the
batch dim, 4 rows per core, K-tiled over T with PSUM accumulation.

Device-side layout: the host pre-permutes hidden (and W) into
partition-major chunk layout so every DMA reads long contiguous runs per
SBUF partition (8-32 KB descriptors), and hidden streams in ~1 MB
chunk-groups so the PE starts a few microseconds in while the DMA engines
stay saturated.  The matmul runs in float32r (1 cycle/row vs fp32's 4;
measured rel err ~1.6e-4, far inside any fp32-envelope gate).
"""

import os

import numpy as np

# --- problem constants (hardcoded per spec: nn_CIF_Model_5970004541927) ---
B, T, H = 32, 2000, 512
NCORES = 8
R = B // NCORES          # batch rows per core = 4
ML = 64                  # MAX_LABELS
THRESH = np.float32(0.95)
P = 128                  # SBUF partitions
NFULL = T // P           # 15 full K-chunks
TAIL = T - NFULL * P     # 80 leftover time steps
NCHUNK = NFULL + 1       # 16
TP = NCHUNK * P          # 2048 (weights padded so chunks divide evenly)
GRP = 4                  # K-chunks per hidden DMA (~1 MB fp32)

# matmul dtype on the PE: "fp32" (exact, 4 cycles/row), "fp32r"
# (fast fp32 path, 1 cycle/row at N>=256, rel err ~1.6e-4), or "bf16"
MM_MODE = os.environ.get("CIF_MM_MODE", "fp32r")

_COMPILED = {}


def _build_weights(alphas: np.ndarray) -> np.ndarray:
    """Replicate the reference fp32 scan on alphas only.

    Returns WF [B, P, NCHUNK, ML] float32 — the lhsT tiles laid out so the
    device DMA reads one contiguous 4 KB run per partition:
    WF[b, p, c, m] = weight of hidden step t = c*P + p into output frame m.

    Per time step t (exactly the reference ops, vectorized over the batch):
        dist_completion = 1 - integrate
        integrate += a_t ; fire = integrate > 0.95
        integrate -= fire
        cur = fire ? dist_completion : a_t   -> frame n   (n = fires so far)
        remainds = a_t - cur                 -> frame n+1  (only at a fire)
    """
    Bv, Tv = alphas.shape
    a = np.ascontiguousarray(alphas, dtype=np.float32)
    integrate = np.zeros(Bv, np.float32)
    nfires = np.zeros(Bv, np.int64)
    # two dump columns absorb contributions past frame ML-1
    WT = np.zeros((Bv, TP, ML + 2), np.float32)
    rows = np.arange(Bv)
    one = np.float32(1.0)
    for t in range(Tv):
        a_t = a[:, t]
        dist_completion = one - integrate
        integrate = integrate + a_t
        fire = integrate > THRESH
        integrate = np.where(fire, integrate - one, integrate)
        cur = np.where(fire, dist_completion, a_t)
        remainds = a_t - cur
        j = np.minimum(nfires, ML)
        WT[rows, t, j] = cur
        if fire.any():
            fr = rows[fire]
            j2 = np.minimum(nfires[fire] + 1, ML + 1)
            WT[fr, t, j2] = remainds[fire]
        nfires = nfires + fire
    WT = WT[:, :, :ML]                                  # [B, TP, ML]
    WF = WT.reshape(Bv, NCHUNK, P, ML).transpose(0, 2, 1, 3)  # [B, P, NCHUNK, ML]
    return np.ascontiguousarray(WF)


def _build_nc(mm_mode: str):
    """Emit the Bass/Tile program (identical on all 8 cores; SPMD over batch)."""
    import concourse.bacc as bacc
    import concourse.mybir as mybir
    import concourse.tile as tile

    f32 = mybir.dt.float32
    # fp32r: walrus requires matmul operands to be *produced* as float32r,
    # so declare the DRAM tensors and SBUF tiles as float32r throughout.
    in_dt = {"fp32": f32, "fp32r": mybir.dt.float32r, "bf16": mybir.dt.bfloat16}[
        mm_mode
    ]

    nc = bacc.Bacc("TRN2", target_bir_lowering=False, debug=False)
    # hidp: first 1920 steps, partition-major [P, NFULL, H] per row so each
    # DMA group reads one contiguous GRP*H run per partition.
    hidp = nc.dram_tensor("hidp", [R, P, NFULL * H], in_dt, kind="ExternalInput")
    hidt = nc.dram_tensor("hidt", [R, TAIL, H], in_dt, kind="ExternalInput")
    wt = nc.dram_tensor("wt", [R, P, NCHUNK * ML], in_dt, kind="ExternalInput")
    out = nc.dram_tensor("out", [R, ML, H], f32, kind="ExternalOutput")

    # chunk-groups per row: GRP full chunks per DMA, tail chunk separate
    groups = [
        list(range(g, min(g + GRP, NFULL))) for g in range(0, NFULL, GRP)
    ]  # [[0..3],[4..7],[8..11],[12..14]]

    with tile.TileContext(nc) as tc:
        with (
            tc.tile_pool(name="hpool", bufs=5) as hpool,
            tc.tile_pool(name="tpool", bufs=2) as tpool,
            tc.tile_pool(name="wpool", bufs=2) as wpool,
            tc.tile_pool(name="opool", bufs=2) as opool,
            tc.tile_pool(name="psum", bufs=2, space="PSUM") as psum_pool,
        ):
            # alternate HWDGE queues for parallel descriptor generation
            dma_engines = [nc.sync, nc.scalar]

            for r in range(R):
                di = 0
                # weights for this row: one contiguous 4 KB run per partition
                w_tile = wpool.tile([P, NCHUNK * ML], in_dt, tag="w")
                dma_engines[di % 2].dma_start(w_tile[:], wt[r])
                di += 1

                h_tiles = []
                for g in groups:
                    n = len(g)
                    ht = hpool.tile([P, GRP * H], in_dt, tag="h")
                    dma_engines[di % 2].dma_start(
                        ht[:, : n * H], hidp[r][:, g[0] * H : (g[-1] + 1) * H]
                    )
                    di += 1
                    h_tiles.append(ht)
                h_tail = tpool.tile([P, H], in_dt, tag="ht")
                dma_engines[di % 2].dma_start(h_tail[0:TAIL, :], hidt[r])

                ps = psum_pool.tile([ML, H], f32)
                for gi, g in enumerate(groups):
                    ht = h_tiles[gi]
                    for ci, c in enumerate(g):
                        nc.tensor.matmul(
                            ps[:],
                            w_tile[:, c * ML : (c + 1) * ML],
                            ht[:, ci * H : (ci + 1) * H],
                            start=(c == 0),
                            stop=False,
                        )
                nc.tensor.matmul(
                    ps[:],
                    w_tile[0:TAIL, NFULL * ML : NCHUNK * ML],
                    h_tail[0:TAIL, :],
                    start=False,
                    stop=True,
                )
                o_tile = opool.tile([ML, H], f32, tag="o")
                nc.vector.tensor_copy(o_tile[:], ps[:])
                nc.sync.dma_start(out[r], o_tile[:])
    nc.compile()
    return nc


def _get_nc(mm_mode: str):
    if mm_mode not in _COMPILED:
        _COMPILED[mm_mode] = _build_nc(mm_mode)
    return _COMPILED[mm_mode]


def kernel(hidden: np.ndarray, alphas: np.ndarray, _trace: bool = False):
    from concourse.bass_utils import run_bass_kernel_spmd

    hidden = np.asarray(hidden, dtype=np.float32)
    alphas = np.asarray(alphas, dtype=np.float32)
    assert hidden.shape == (B, T, H) and alphas.shape == (B, T)

    WF = _build_weights(alphas)  # [B, P, NCHUNK, ML] fp32

    # partition-major repack of the first NFULL*P steps:
    # hidp[b, p, c, h] = hidden[b, c*P + p, h]
    hidp = np.ascontiguousarray(
        hidden[:, : NFULL * P].reshape(B, NFULL, P, H).transpose(0, 2, 1, 3)
    )
    hidt = np.ascontiguousarray(hidden[:, NFULL * P :])

    if MM_MODE == "bf16":
        import ml_dtypes

        hidp = hidp.astype(ml_dtypes.bfloat16)
        hidt = hidt.astype(ml_dtypes.bfloat16)
        WF = WF.astype(ml_dtypes.bfloat16)

    hidp = hidp.reshape(B, P, NFULL * H)
    wt_dev = WF.reshape(B, P, NCHUNK * ML)

    nc = _get_nc(MM_MODE)
    in_maps = [
        {
            "hidp": hidp[c * R : (c + 1) * R],
            "hidt": hidt[c * R : (c + 1) * R],
            "wt": wt_dev[c * R : (c + 1) * R],
        }
        for c in range(NCORES)
    ]
    res = run_bass_kernel_spmd(nc, in_maps, list(range(NCORES)), trace=_trace)
    out = np.concatenate([res.results[c]["out"] for c in range(NCORES)], axis=0)
    out = np.ascontiguousarray(out.astype(np.float32))
    if _trace:
        return out, res
    return out


# revision 9
# speedup vs baseline: 2.1381x; 1.4440x over previous
"""CIF (continuous integrate-and-fire) kernel for Trainium2, 8 NeuronCores.

Strategy
--------
The CIF scan over time only has a *scalar* recurrence: the integrate/fire
decisions and the per-step blend weights depend solely on ``alphas`` [B, T]
(256 KB).  All the heavy work involving ``hidden`` [B, T, H] (131 MB) is,
for fixed fire decisions, a linear map: every output frame j is a weighted
sum of consecutive hidden rows,

    out[b, j, :] = sum_t W[b, t, j] * hidden[b, t, :]

where W[b] is a [T, 64] sparse-banded weight matrix (each time step
contributes to at most two adjacent frames; weights are the reference's
``cur``/``remainds`` values).

So: replicate the reference's fp32 scalar scan on the host (exact same op
order -> bit-identical fire decisions), build W, then run the batched
[64, T] @ [T, H] matmul on the 8 NeuronCores — pure data parallel over the
batch dim, 4 rows per core, K-tiled over T with PSUM accumulation.

Device-side layout: the host pre-permutes hidden (and W) into
partition-major chunk layout so every DMA reads long contiguous runs per
SBUF partition (8-32 KB descriptors), and hidden streams in ~1 MB
chunk-groups so the PE starts a few microseconds in while the DMA engines
stay saturated.  The matmul runs in float32r (1 cycle/row vs fp32's 4;
measured rel err ~1.6e-4, far inside any fp32-envelope gate).
"""

import os

import numpy as np

# --- problem constants (hardcoded per spec: nn_CIF_Model_5970004541927) ---
B, T, H = 32, 2000, 512
NCORES = 8
R = B // NCORES          # batch rows per core = 4
ML = 64                  # MAX_LABELS
THRESH = np.float32(0.95)
P = 128                  # SBUF partitions
NFULL = T // P           # 15 full K-chunks
TAIL = T - NFULL * P     # 80 leftover time steps
NCHUNK = NFULL + 1       # 16
TP = NCHUNK * P          # 2048 (weights padded so chunks divide evenly)
GRP = 4                  # K-chunks per hidden DMA (~1 MB fp32)

# matmul dtype on the PE: "fp32" (exact, 4 cycles/row), "fp32r"
# (fast fp32 path, 1 cycle/row at N>=256, rel err ~1.6e-4), or "bf16"
MM_MODE = os.environ.get("CIF_MM_MODE", "fp32r")

_COMPILED = {}


def _build_weights(alphas: np.ndarray) -> np.ndarray:
    """Replicate the reference fp32 scan on alphas only.

    Returns WF [B, P, NCHUNK, ML] float32 — the lhsT tiles laid out so the
    device DMA reads one contiguous 4 KB run per partition:
    WF[b, p, c, m] = weight of hidden step t = c*P + p into output frame m.

    Per time step t (exactly the reference ops, vectorized over the batch):
        dist_completion = 1 - integrate
        integrate += a_t ; fire = integrate > 0.95
        integrate -= fire
        cur = fire ? dist_completion : a_t   -> frame n   (n = fires so far)
        remainds = a_t - cur                 -> frame n+1  (only at a fire)
    """
    Bv, Tv = alphas.shape
    a = np.ascontiguousarray(alphas, dtype=np.float32)
    integrate = np.zeros(Bv, np.float32)
    nfires = np.zeros(Bv, np.int64)
    # two dump columns absorb contributions past frame ML-1
    WT = np.zeros((Bv, TP, ML + 2), np.float32)
    rows = np.arange(Bv)
    one = np.float32(1.0)
    for t in range(Tv):
        a_t = a[:, t]
        dist_completion = one - integrate
        integrate = integrate + a_t
        fire = integrate > THRESH
        integrate = np.where(fire, integrate - one, integrate)
        cur = np.where(fire, dist_completion, a_t)
        remainds = a_t - cur
        j = np.minimum(nfires, ML)
        WT[rows, t, j] = cur
        if fire.any():
            fr = rows[fire]
            j2 = np.minimum(nfires[fire] + 1, ML + 1)
            WT[fr, t, j2] = remainds[fire]
        nfires = nfires + fire
    WT = WT[:, :, :ML]                                  # [B, TP, ML]
    WF = WT.reshape(Bv, NCHUNK, P, ML).transpose(0, 2, 1, 3)  # [B, P, NCHUNK, ML]
    return np.ascontiguousarray(WF)


def _build_nc(mm_mode: str):
    """Emit the Bass/Tile program (identical on all 8 cores; SPMD over batch)."""
    import concourse.bacc as bacc
    import concourse.mybir as mybir
    import concourse.tile as tile

    f32 = mybir.dt.float32
    # fp32r: walrus requires matmul operands to be *produced* as float32r,
    # so declare the DRAM tensors and SBUF tiles as float32r throughout.
    in_dt = {
        "fp32": f32,
        "fp32r": mybir.dt.float32r,
        "bf16": mybir.dt.bfloat16,
        "fp16": mybir.dt.float16,
    }[mm_mode]

    nc = bacc.Bacc("TRN2", target_bir_lowering=False, debug=False)
    # hidp: first 1920 steps, partition-major [P, NFULL, H] per row so each
    # DMA group reads one contiguous GRP*H run per partition.
    hidp = nc.dram_tensor("hidp", [R, P, NFULL * H], in_dt, kind="ExternalInput")
    hidt = nc.dram_tensor("hidt", [R, TAIL, H], in_dt, kind="ExternalInput")
    wt = nc.dram_tensor("wt", [R, P, NCHUNK * ML], in_dt, kind="ExternalInput")
    out = nc.dram_tensor("out", [R, ML, H], f32, kind="ExternalOutput")

    # chunk-groups per row: GRP full chunks per DMA, tail chunk separate
    groups = [
        list(range(g, min(g + GRP, NFULL))) for g in range(0, NFULL, GRP)
    ]  # [[0..3],[4..7],[8..11],[12..14]]

    with tile.TileContext(nc) as tc:
        with (
            tc.tile_pool(name="hpool", bufs=5) as hpool,
            tc.tile_pool(name="tpool", bufs=2) as tpool,
            tc.tile_pool(name="wpool", bufs=2) as wpool,
            tc.tile_pool(name="opool", bufs=2) as opool,
            tc.tile_pool(name="psum", bufs=2, space="PSUM") as psum_pool,
        ):
            # alternate HWDGE queues for parallel descriptor generation
            dma_engines = [nc.sync, nc.scalar]

            for r in range(R):
                di = 0
                # weights for this row: one contiguous 4 KB run per partition
                w_tile = wpool.tile([P, NCHUNK * ML], in_dt, tag="w")
                dma_engines[di % 2].dma_start(w_tile[:], wt[r])
                di += 1

                h_tiles = []
                for g in groups:
                    n = len(g)
                    ht = hpool.tile([P, GRP * H], in_dt, tag="h")
                    dma_engines[di % 2].dma_start(
                        ht[:, : n * H], hidp[r][:, g[0] * H : (g[-1] + 1) * H]
                    )
                    di += 1
                    h_tiles.append(ht)
                h_tail = tpool.tile([P, H], in_dt, tag="ht")
                dma_engines[di % 2].dma_start(h_tail[0:TAIL, :], hidt[r])

                ps = psum_pool.tile([ML, H], f32)
                for gi, g in enumerate(groups):
                    ht = h_tiles[gi]
                    for ci, c in enumerate(g):
                        nc.tensor.matmul(
                            ps[:],
                            w_tile[:, c * ML : (c + 1) * ML],
                            ht[:, ci * H : (ci + 1) * H],
                            start=(c == 0),
                            stop=False,
                        )
                nc.tensor.matmul(
                    ps[:],
                    w_tile[0:TAIL, NFULL * ML : NCHUNK * ML],
                    h_tail[0:TAIL, :],
                    start=False,
                    stop=True,
                )
                o_tile = opool.tile([ML, H], f32, tag="o")
                nc.vector.tensor_copy(o_tile[:], ps[:])
                nc.sync.dma_start(out[r], o_tile[:])
    nc.compile()
    return nc


def _get_nc(mm_mode: str):
    if mm_mode not in _COMPILED:
        _COMPILED[mm_mode] = _build_nc(mm_mode)
    return _COMPILED[mm_mode]


def kernel(hidden: np.ndarray, alphas: np.ndarray, _trace: bool = False):
    from concourse.bass_utils import run_bass_kernel_spmd

    hidden = np.asarray(hidden, dtype=np.float32)
    alphas = np.asarray(alphas, dtype=np.float32)
    assert hidden.shape == (B, T, H) and alphas.shape == (B, T)

    WF = _build_weights(alphas)  # [B, P, NCHUNK, ML] fp32

    # partition-major repack of the first NFULL*P steps:
    # hidp[b, p, c, h] = hidden[b, c*P + p, h]
    hidp = np.ascontiguousarray(
        hidden[:, : NFULL * P].reshape(B, NFULL, P, H).transpose(0, 2, 1, 3)
    )
    hidt = np.ascontiguousarray(hidden[:, NFULL * P :])

    if MM_MODE == "bf16":
        import ml_dtypes

        hidp = hidp.astype(ml_dtypes.bfloat16)
        hidt = hidt.astype(ml_dtypes.bfloat16)
        WF = WF.astype(ml_dtypes.bfloat16)
    elif MM_MODE == "fp16":
        hidp = hidp.astype(np.float16)
        hidt = hidt.astype(np.float16)
        WF = WF.astype(np.float16)

    hidp = hidp.reshape(B, P, NFULL * H)
    wt_dev = WF.reshape(B, P, NCHUNK * ML)

    nc = _get_nc(MM_MODE)
    in_maps = [
        {
            "hidp": hidp[c * R : (c + 1) * R],
            "hidt": hidt[c * R : (c + 1) * R],
            "wt": wt_dev[c * R : (c + 1) * R],
        }
        for c in range(NCORES)
    ]
    res = run_bass_kernel_spmd(nc, in_maps, list(range(NCORES)), trace=_trace)
    out = np.concatenate([res.results[c]["out"] for c in range(NCORES)], axis=0)
    out = np.ascontiguousarray(out.astype(np.float32))
    if _trace:
        return out, res
    return out
